# revision 1
# baseline (speedup 1.0000x reference)
"""Single-head causal attention (B=16, T=2048, C=1024, D=128) on 8 TRN2 cores.

Data-parallel over batch: each core handles 2 batches. Inside each core:
  xT = transpose(x) via PE transposes           [C on partitions]
  qT/kT/vT = W.T @ xT  (fp32r matmuls)          [D on partitions, T free]
  V = transpose(vT)                             [Tk on partitions, D free]
  per 512-wide query block, per 128-key tile:
    S^T tile = kT_tile.T @ qT_block             [Tk part, Tq free] (PSUM)
    + causal mask on diagonal tiles (DVE add)
    P^T = exp(scale * S^T)  (ACT, writes f32r SBUF)
    OT  += V_tile.T @ P^T                       [D part, Tq free]
    rsum += ones.T @ P^T                        [1, Tq]
  rsumT = tiny transpose matmuls -> [Tq part, 1] ; recip on DVE
  O = transpose(OT) normalized during PSUM evac by recipT (tensor_scalar_mul)
"""

import numpy as np

import concourse.bacc as bacc
import concourse.mybir as mybir
import concourse.tile as tile
from concourse.bass_utils import run_bass_kernel_spmd
from concourse.masks import make_identity

F32 = mybir.dt.float32
F32R = mybir.dt.float32r

B, T, C, D = 16, 2048, 1024, 128
NCORES = 8
BLOC = B // NCORES  # batches per core
NBLK = T // 512  # query blocks of width 512
NKT = T // 128  # key tiles of 128
SCALE = float(D) ** -0.5


def _build():
    nc = bacc.Bacc("TRN2", target_bir_lowering=False, debug=False, num_devices=NCORES)
    x_d = nc.dram_tensor("x", [BLOC, T, C], F32, kind="ExternalInput").ap()
    wq_d = nc.dram_tensor("Wq", [C, D], F32, kind="ExternalInput").ap()
    wk_d = nc.dram_tensor("Wk", [C, D], F32, kind="ExternalInput").ap()
    wv_d = nc.dram_tensor("Wv", [C, D], F32, kind="ExternalInput").ap()
    out_d = nc.dram_tensor("out", [BLOC, T, D], F32, kind="ExternalOutput").ap()

    with tile.TileContext(nc) as tc:
        _emit(nc, tc, x_d, (wq_d, wk_d, wv_d), out_d)
    nc.compile()
    return nc


def _emit(nc, tc, x_d, w_ds, out_d):
    from contextlib import ExitStack

    ctx = ExitStack()
    with ctx:
        const = ctx.enter_context(tc.tile_pool(name="const", bufs=1))
        xtp = ctx.enter_context(tc.tile_pool(name="xtp", bufs=1))
        stage = ctx.enter_context(tc.tile_pool(name="stage", bufs=3))
        qkv = ctx.enter_context(tc.tile_pool(name="qkv", bufs=2))
        ppool = ctx.enter_context(tc.tile_pool(name="ppool", bufs=12))
        small = ctx.enter_context(tc.tile_pool(name="small", bufs=3))
        ps_t = ctx.enter_context(tc.tile_pool(name="ps_t", bufs=1, space="PSUM"))
        ps_proj = ctx.enter_context(tc.tile_pool(name="ps_proj", bufs=2, space="PSUM"))
        ps_s = ctx.enter_context(tc.tile_pool(name="ps_s", bufs=3, space="PSUM"))
        ps_ot = ctx.enter_context(tc.tile_pool(name="ps_ot", bufs=1, space="PSUM"))
        ps_rs = ctx.enter_context(tc.tile_pool(name="ps_rs", bufs=1, space="PSUM"))

        # ---- constants ----
        ident = const.tile([128, 128], F32, tag="ident")
        make_identity(nc, ident)
        masks = const.tile([128, 4, 512], F32, tag="masks")
        nc.gpsimd.memset(masks, 0.0)
        for i in range(4):
            # valid (keep 0.0) iff q - k - 128*i >= 0 ; else fill -1e30
            nc.gpsimd.affine_select(
                out=masks[:, i, :],
                in_=masks[:, i, :],
                compare_op=mybir.AluOpType.is_ge,
                fill=-1e30,
                base=-128 * i,
                pattern=[[1, 512]],
                channel_multiplier=-1,
            )
        ones_f32 = const.tile([128, 1], F32, tag="ones_f32")
        nc.vector.memset(ones_f32, 1.0)
        ones_col = const.tile([128, 1], F32R, tag="ones")
        nc.vector.tensor_copy(ones_col, ones_f32)
        one_one = const.tile([1, 1], F32, tag="oneone")
        nc.vector.memset(one_one, 1.0)
        w_t = const.tile([128, 3, 8, 128], F32R, tag="w")
        for p in range(3):
            nc.sync.dma_start(
                out=w_t[:, p],
                in_=w_ds[p].bitcast(F32R).rearrange("(k p) d -> p k d", p=128),
            )

        # evac engine round-robin (PSUM -> SBUF copies)
        evac_state = [0]

        def evac(out_ap, in_ap):
            if evac_state[0] % 2 == 0:
                nc.vector.tensor_copy(out_ap, in_ap)
            else:
                nc.scalar.copy(out_ap, in_ap)
            evac_state[0] += 1

        for b in range(BLOC):
            # ---- phase X: load x and transpose to xT [C-part, T] ----
            xT = xtp.tile([128, 8, T], F32R, tag="xT")
            for g in range(T // 512):
                for cc in range(8):
                    st = stage.tile([128, 4, 128], F32, tag="stage")
                    nc.sync.dma_start(
                        out=st,
                        in_=x_d[
                            b, 512 * g : 512 * (g + 1), 128 * cc : 128 * (cc + 1)
                        ].rearrange("(ts p) c -> p ts c", p=128),
                    )
                    tp = ps_t.tile([128, 512], F32, tag="tpose")
                    for ts in range(4):
                        nc.tensor.transpose(
                            tp[:, 128 * ts : 128 * (ts + 1)], st[:, ts, :], ident
                        )
                    evac(xT[:, cc, 512 * g : 512 * (g + 1)], tp)

            # ---- phase P: projections qT/kT [D-part, T], V [Tk-part, D] ----
            qT = qkv.tile([128, T], F32R, tag="qT")
            kT = qkv.tile([128, T], F32R, tag="kT")
            V = qkv.tile([128, NKT, 128], F32R, tag="V")
            for j in range(NBLK):
                sl = slice(512 * j, 512 * (j + 1))
                for p, dst in ((0, qT), (1, kT), (2, None)):
                    acc = ps_proj.tile([128, 512], F32, tag="proj")
                    for kk in range(8):
                        nc.tensor.matmul(
                            acc,
                            w_t[:, p, kk],
                            xT[:, kk, sl],
                            start=(kk == 0),
                            stop=(kk == 7),
                        )
                    if dst is not None:
                        evac(dst[:, sl], acc)
                    else:
                        vt_tmp = small.tile([128, 512], F32, tag="vt")
                        evac(vt_tmp, acc)
                        vp = ps_t.tile([128, 512], F32, tag="tpose")
                        for m in range(4):
                            nc.tensor.transpose(
                                vp[:, 128 * m : 128 * (m + 1)],
                                vt_tmp[:, 128 * m : 128 * (m + 1)],
                                ident,
                            )
                        evac(V[:, 4 * j : 4 * (j + 1)].rearrange("p m d -> p (m d)"), vp)

            # ---- phase A: attention ----
            for j in range(NBLK):
                sl = slice(512 * j, 512 * (j + 1))
                ntk = 4 * (j + 1)
                ot = ps_ot.tile([128, 512], F32, tag="ot")
                rs = ps_rs.tile([1, 512], F32, tag="rs")
                for tk in range(ntk):
                    sp = ps_s.tile([128, 512], F32, tag="s")
                    nc.tensor.matmul(
                        sp,
                        kT[:, 128 * tk : 128 * (tk + 1)],
                        qT[:, sl],
                        start=True,
                        stop=True,
                    )
                    if tk >= 4 * j:
                        nc.vector.tensor_tensor(
                            sp, sp, masks[:, tk - 4 * j, :], mybir.AluOpType.add
                        )
                    pt = ppool.tile([128, 512], F32R, tag="p")
                    nc.scalar.activation(
                        pt, sp, mybir.ActivationFunctionType.Exp, scale=SCALE
                    )
                    nc.tensor.matmul(
                        ot, V[:, tk], pt, start=(tk == 0), stop=(tk == ntk - 1)
                    )
                    nc.tensor.matmul(
                        rs, ones_col, pt, start=(tk == 0), stop=(tk == ntk - 1)
                    )
                # rowsum -> transposed reciprocal
                rs_sb = small.tile([1, 512], F32, tag="rssb")
                nc.vector.tensor_copy(rs_sb, rs)
                rsT = ps_rs.tile([128, 4], F32, tag="rs")
                for t in range(4):
                    nc.tensor.matmul(
                        rsT[:, t : t + 1],
                        rs_sb[0:1, 128 * t : 128 * (t + 1)],
                        one_one,
                        start=True,
                        stop=True,
                    )
                recipT = small.tile([128, 4], F32, tag="recip")
                nc.vector.reciprocal(recipT, rsT)
                # OT -> SBUF, transpose to natural layout, normalize, DMA out
                ot_sb = small.tile([128, 512], F32, tag="otsb")
                evac(ot_sb, ot)
                op = ps_t.tile([128, 512], F32, tag="tpose")
                for t in range(4):
                    nc.tensor.transpose(
                        op[:, 128 * t : 128 * (t + 1)],
                        ot_sb[:, 128 * t : 128 * (t + 1)],
                        ident,
                    )
                o_sb = small.tile([128, 4, 128], F32, tag="osb")
                for t in range(4):
                    nc.vector.tensor_scalar_mul(
                        o_sb[:, t, :],
                        op[:, 128 * t : 128 * (t + 1)],
                        recipT[:, t : t + 1],
                    )
                nc.sync.dma_start(
                    out=out_d[b, sl, :].rearrange("(t p) d -> p t d", p=128),
                    in_=o_sb,
                )


_NC = None


def _get_nc():
    global _NC
    if _NC is None:
        _NC = _build()
    return _NC


def kernel(x, Wq, Wk, Wv):
    nc = _get_nc()
    x = np.ascontiguousarray(x, dtype=np.float32)
    in_maps = [
        {"x": x[BLOC * c : BLOC * (c + 1)], "Wq": Wq, "Wk": Wk, "Wv": Wv}
        for c in range(NCORES)
    ]
    res = run_bass_kernel_spmd(nc, in_maps, core_ids=list(range(NCORES)))
    return np.concatenate([res.results[c]["out"] for c in range(NCORES)], axis=0)



# revision 9
# speedup vs baseline: 1.0957x; 1.0957x over previous
"""Single-head causal attention (B=16, T=2048, C=1024, D=128) on 8 TRN2 cores.

Data-parallel over batch: each core handles 2 batches. Inputs are cast to
bf16 on the host so x can be loaded pre-transposed via the DMA xbar
transpose (no PE transposes anywhere).

Per core, per batch:
  xT = dma_transpose(x)                      [C on partitions, bf16]
  qT/kT = W.T @ xT                           [D part, T free]
  V     = xT.T @ Wv  (per 128-t chunk)       [T part, D free]  (natural)
  attention over 512-wide q blocks j, 128-wide k tiles:
    sub-diagonal tiles (tk < 4j, fully valid):
      S^T = kT_tile.T @ qT_block             [k part, 512 q]  (PSUM)
      P^T = exp(scale*S^T)  -> bf16 SBUF     (feeds AV directly)
      rowsum partial = gpsimd partition-reduce(P^T) -> [1, 512]
      partials accumulated into rsT [q,1] via 1-row PE matmuls (PSUM acc)
    diagonal wedge (tk in [4j, gq], per 128-q chunk): S natural [q, k],
      exp -> P nat bf16, causal mask via gpsimd affine_select (fill=0),
      rowsum via DVE free-axis reduce, P^T via SBUF->SBUF DMA transpose
    O = sum_tk P^T_tile.T @ V_tile           [q part, D free]  (natural)
    O normalized by 1/rowsum during PSUM evac, DMA out (f32)
"""

import os

import numpy as np
import ml_dtypes

DEBUG_DEN = bool(os.environ.get("KDEBUG_DEN"))

import concourse.bacc as bacc
import concourse.mybir as mybir
import concourse.tile as tile
from concourse.bass_utils import run_bass_kernel_spmd

F32 = mybir.dt.float32
BF16 = mybir.dt.bfloat16

B, T, C, D = 16, 2048, 1024, 128
NCORES = 8
BLOC = B // NCORES  # batches per core
NJ = T // 512  # 512-wide q blocks
NKT = T // 128  # 128-wide k tiles
SCALE = float(D) ** -0.5


def _build():
    nc = bacc.Bacc("TRN2", target_bir_lowering=False, debug=False, num_devices=NCORES)
    x_d = nc.dram_tensor("x", [BLOC, T, C], BF16, kind="ExternalInput").ap()
    wq_d = nc.dram_tensor("Wq", [C, D], BF16, kind="ExternalInput").ap()
    wk_d = nc.dram_tensor("Wk", [C, D], BF16, kind="ExternalInput").ap()
    wv_d = nc.dram_tensor("Wv", [C, D], BF16, kind="ExternalInput").ap()
    out_d = nc.dram_tensor("out", [BLOC, T, D], F32, kind="ExternalOutput").ap()
    dbg = None
    if DEBUG_DEN:
        den_d = nc.dram_tensor("den", [BLOC, 16, 128], F32, kind="ExternalOutput").ap()
        rsub_d = nc.dram_tensor("rsub", [BLOC, 16, 128], F32, kind="ExternalOutput").ap()
        dbg = (den_d, rsub_d)

    with tile.TileContext(nc) as tc:
        _emit(nc, tc, x_d, (wq_d, wk_d, wv_d), out_d, dbg)
    nc.compile()
    return nc


def _emit(nc, tc, x_d, w_ds, out_d, dbg=None):
    from contextlib import ExitStack

    ctx = ExitStack()
    with ctx:
        const = ctx.enter_context(tc.tile_pool(name="const", bufs=1))
        xtp = ctx.enter_context(tc.tile_pool(name="xtp", bufs=2))
        qkv = ctx.enter_context(tc.tile_pool(name="qkv", bufs=2))
        ptsub = ctx.enter_context(tc.tile_pool(name="ptsub", bufs=2))
        ptnat = ctx.enter_context(tc.tile_pool(name="ptnat", bufs=3))
        ptwed = ctx.enter_context(tc.tile_pool(name="ptwed", bufs=3))
        part = ctx.enter_context(tc.tile_pool(name="part", bufs=2))
        small = ctx.enter_context(tc.tile_pool(name="small", bufs=4))
        ps_proj = ctx.enter_context(tc.tile_pool(name="ps_proj", bufs=2, space="PSUM"))
        ps_s = ctx.enter_context(tc.tile_pool(name="ps_s", bufs=3, space="PSUM"))
        ps_o = ctx.enter_context(tc.tile_pool(name="ps_o", bufs=2, space="PSUM"))
        ps_rs = ctx.enter_context(tc.tile_pool(name="ps_rs", bufs=1, space="PSUM"))

        # ---- constants ----
        one_one = const.tile([1, 1], F32, tag="oneone")
        nc.vector.memset(one_one, 1.0)
        w_t = const.tile([128, 3, 8, 128], BF16, tag="w")
        for p in range(3):
            nc.sync.dma_start(
                out=w_t[:, p], in_=w_ds[p].rearrange("(k p) d -> p k d", p=128)
            )

        # evac engine round-robin (PSUM -> SBUF copies / scaled copies)
        evac_state = [0]

        def evac(out_ap, in_ap):
            if evac_state[0] % 2 == 0:
                nc.vector.tensor_copy(out_ap, in_ap)
            else:
                nc.scalar.copy(out_ap, in_ap)
            evac_state[0] += 1

        def norm_evac(out_ap, in_ap, recip_ap):
            if evac_state[0] % 2 == 0:
                nc.vector.tensor_scalar_mul(out_ap, in_ap, recip_ap)
            else:
                nc.scalar.activation(
                    out_ap,
                    in_ap,
                    mybir.ActivationFunctionType.Copy,
                    scale=recip_ap,
                )
            evac_state[0] += 1

        def emit_xload(b):
            """xT [128, 8, T] bf16 via DMA xbar transpose, 4 slabs."""
            xT = xtp.tile([128, 8, T], BF16, tag="xT")
            for s in range(4):
                sl = slice(512 * s, 512 * (s + 1))
                nc.sync.dma_start_transpose(xT[:, :, sl], x_d[b, sl, :])
            return xT

        def emit_proj_slab(xT, qkv_t, j):
            """Projections for 512-wide t slab j."""
            qT, kT, V = qkv_t
            sl = slice(512 * j, 512 * (j + 1))
            for p, dst in ((0, qT), (1, kT)):
                acc = ps_proj.tile([128, 512], F32, tag="proj")
                for kk in range(8):
                    nc.tensor.matmul(
                        acc,
                        w_t[:, p, kk],
                        xT[:, kk, sl],
                        start=(kk == 0),
                        stop=(kk == 7),
                    )
                evac(dst[:, sl], acc)
            # V natural: per 128-t chunk within the slab
            acc = ps_proj.tile([128, 512], F32, tag="proj")
            for m in range(4):
                tch = 4 * j + m
                for kk in range(8):
                    nc.tensor.matmul(
                        acc[:, 128 * m : 128 * (m + 1)],
                        xT[:, kk, 128 * tch : 128 * (tch + 1)],
                        w_t[:, 2, kk],
                        start=(kk == 0),
                        stop=(kk == 7),
                    )
            evac(V[:, 4 * j : 4 * (j + 1)].rearrange("p m d -> p (m d)"), acc)

        def emit_attn_block(b, qkv_t, j):
            qT, kT, V = qkv_t
            nsub = 4 * j  # fully-valid k tiles for this 512-q block
            qsl = slice(512 * j, 512 * (j + 1))

            # ---- sub-diagonal tiles in S^T form ----
            if nsub:
                ptT_sub = ptsub.tile([128, 12, 512], BF16, tag="ptsub")
                partial = part.tile([1, 12, 512], F32, tag="partial")
                rsT = ps_rs.tile([128, 4], F32, tag="rsT")
            for tk in range(nsub):
                sp = ps_s.tile([128, 512], F32, tag="s")
                nc.tensor.matmul(
                    sp,
                    kT[:, 128 * tk : 128 * (tk + 1)],
                    qT[:, qsl],
                    start=True,
                    stop=True,
                )
                nc.scalar.activation(
                    ptT_sub[:, tk, :],
                    sp,
                    mybir.ActivationFunctionType.Exp,
                    scale=SCALE,
                )
                nc.gpsimd.tensor_reduce(
                    out=partial[:, tk, :],
                    in_=ptT_sub[:, tk, :],
                    axis=mybir.AxisListType.C,
                    op=mybir.AluOpType.add,
                )
                # transpose partial into rsT[:, qq] with PSUM accumulation.
                # NOTE: start=True zeroes the full psum zero-region (covers all
                # 4 columns), so exactly one start/stop for the whole chain.
                for qq in range(4):
                    nc.tensor.matmul(
                        rsT[:, qq : qq + 1],
                        partial[0:1, tk, 128 * qq : 128 * (qq + 1)],
                        one_one,
                        start=(tk == 0 and qq == 0),
                        stop=(tk == nsub - 1 and qq == 3),
                    )

            # ---- diagonal wedge, natural [q, k] layout per 128-q chunk ----
            wedges = []
            for qq in range(4):
                gq = 4 * j + qq
                nw = qq + 1  # k tiles in [4j, gq]
                sn = ps_s.tile([128, 512], F32, tag="s")
                for m in range(nw):
                    tk = 4 * j + m
                    nc.tensor.matmul(
                        sn[:, 128 * m : 128 * (m + 1)],
                        qT[:, 128 * gq : 128 * (gq + 1)],
                        kT[:, 128 * tk : 128 * (tk + 1)],
                        start=True,
                        stop=True,
                    )
                ptn = ptnat.tile([128, 512], BF16, tag="ptnat")
                nc.scalar.activation(
                    ptn[:, : 128 * nw],
                    sn[:, : 128 * nw],
                    mybir.ActivationFunctionType.Exp,
                    scale=SCALE,
                )
                # causal mask on the diagonal 128x128 slice: keep q >= k
                nc.gpsimd.affine_select(
                    out=ptn[:, 128 * qq : 128 * (qq + 1)],
                    in_=ptn[:, 128 * qq : 128 * (qq + 1)],
                    compare_op=mybir.AluOpType.is_ge,
                    fill=0.0,
                    base=0,
                    pattern=[[-1, 128]],
                    channel_multiplier=1,
                )
                rsw = small.tile([128, 1], F32, tag="rsw")
                nc.vector.tensor_reduce(
                    out=rsw,
                    in_=ptn[:, : 128 * nw],
                    axis=mybir.AxisListType.X,
                    op=mybir.AluOpType.add,
                )
                ptw = ptwed.tile([128, 4, 128], BF16, tag="ptwed")
                nc.sync.dma_start_transpose(ptw[:, :nw, :], ptn[:, : 128 * nw])
                wedges.append((ptn, ptw, rsw))

            # ---- AV + normalize + store per 128-q chunk ----
            ot = ps_o.tile([128, 512], F32, tag="ot")
            for qq in range(4):
                gq = 4 * j + qq
                _, ptw, rsw = wedges[qq]
                osl = ot[:, 128 * qq : 128 * (qq + 1)]
                nmm = gq + 1
                i = 0
                for tk in range(nsub):
                    nc.tensor.matmul(
                        osl,
                        ptT_sub[:, tk, 128 * qq : 128 * (qq + 1)],
                        V[:, tk, :],
                        start=(i == 0),
                        stop=(i == nmm - 1),
                    )
                    i += 1
                for m in range(qq + 1):
                    nc.tensor.matmul(
                        osl,
                        ptw[:, m, :],
                        V[:, 4 * j + m, :],
                        start=(i == 0),
                        stop=(i == nmm - 1),
                    )
                    i += 1
                # total rowsum -> reciprocal
                recip = small.tile([128, 1], F32, tag="recip")
                if nsub:
                    tot = small.tile([128, 1], F32, tag="tot")
                    nc.vector.tensor_tensor(
                        tot, rsw, rsT[:, qq : qq + 1], mybir.AluOpType.add
                    )
                    nc.vector.reciprocal(recip, tot)
                else:
                    tot = rsw
                    nc.vector.reciprocal(recip, rsw)
                if dbg is not None:
                    den_d, rsub_d = dbg
                    nc.sync.dma_start(
                        out=den_d[b, gq : gq + 1, :].rearrange("one p -> p one"),
                        in_=tot,
                    )
                    rsub_sb = small.tile([128, 1], F32, tag="rsubsb")
                    if nsub:
                        nc.vector.tensor_copy(rsub_sb, rsT[:, qq : qq + 1])
                    else:
                        nc.vector.memset(rsub_sb, 0.0)
                    nc.sync.dma_start(
                        out=rsub_d[b, gq : gq + 1, :].rearrange("one p -> p one"),
                        in_=rsub_sb,
                    )
                o_sb = small.tile([128, 128], F32, tag="osb")
                norm_evac(o_sb, osl, recip)
                nc.sync.dma_start(
                    out=out_d[b, 128 * gq : 128 * (gq + 1), :], in_=o_sb
                )

        # ---------- schedule ----------
        xT0 = emit_xload(0)
        qkv0 = (
            qkv.tile([128, T], BF16, tag="qT", name="qT"),
            qkv.tile([128, T], BF16, tag="kT", name="kT"),
            qkv.tile([128, NKT, 128], BF16, tag="V", name="V"),
        )
        for j in range(NJ):
            emit_proj_slab(xT0, qkv0, j)
        # prefetch batch 1 x while attention(0) runs
        xT1 = emit_xload(1)
        qkv1 = (
            qkv.tile([128, T], BF16, tag="qT", name="qT"),
            qkv.tile([128, T], BF16, tag="kT", name="kT"),
            qkv.tile([128, NKT, 128], BF16, tag="V", name="V"),
        )
        # interleave attention(0) with projections(1)
        for j in range(NJ):
            emit_attn_block(0, qkv0, j)
            emit_proj_slab(xT1, qkv1, j)
        for j in range(NJ):
            emit_attn_block(1, qkv1, j)


_NC = None


def _get_nc():
    global _NC
    if _NC is None:
        _NC = _build()
    return _NC


def _in_maps(x, Wq, Wk, Wv):
    xb = np.ascontiguousarray(x).astype(ml_dtypes.bfloat16)
    wqb = np.ascontiguousarray(Wq).astype(ml_dtypes.bfloat16)
    wkb = np.ascontiguousarray(Wk).astype(ml_dtypes.bfloat16)
    wvb = np.ascontiguousarray(Wv).astype(ml_dtypes.bfloat16)
    return [
        {"x": xb[BLOC * c : BLOC * (c + 1)], "Wq": wqb, "Wk": wkb, "Wv": wvb}
        for c in range(NCORES)
    ]


def kernel(x, Wq, Wk, Wv):
    nc = _get_nc()
    res = run_bass_kernel_spmd(nc, _in_maps(x, Wq, Wk, Wv), core_ids=list(range(NCORES)))
    return np.concatenate([res.results[c]["out"] for c in range(NCORES)], axis=0)


# revision 14
# speedup vs baseline: 1.3044x; 1.1904x over previous
"""Single-head causal attention (B=16, T=2048, C=1024, D=128) on 8 TRN2 cores.

Data-parallel over batch: each core handles 2 batches. Inputs are cast to
bf16 on the host so x can be loaded pre-transposed via the DMA xbar
transpose (no PE transposes anywhere).

Per core, per batch:
  xT = dma_transpose(x)                      [C on partitions, bf16]
  qT/kT = W.T @ xT                           [D part, T free]
  V     = xT.T @ Wv  (per 128-t chunk)       [T part, D free]  (natural)
  attention over 512-wide q blocks j, 128-wide k tiles:
    sub-diagonal tiles (tk < 4j, fully valid):
      S^T = kT_tile.T @ qT_block             [k part, 512 q]  (PSUM)
      P^T = exp(scale*S^T)  -> bf16 SBUF     (feeds AV directly)
      rowsum partial = gpsimd partition-reduce(P^T) -> [1, 512]
      partials accumulated into rsT [q,1] via 1-row PE matmuls (PSUM acc)
    diagonal wedge (tk in [4j, gq], per 128-q chunk): S natural [q, k],
      exp -> P nat bf16, causal mask via gpsimd affine_select (fill=0),
      rowsum via DVE free-axis reduce, P^T via SBUF->SBUF DMA transpose
    O = sum_tk P^T_tile.T @ V_tile           [q part, D free]  (natural)
    O normalized by 1/rowsum during PSUM evac, DMA out (f32)

Schedule: per block, AV is delayed one block (proj(b,j) / pre(b,j) /
av(b,j-1)) so the wedge DMA-transpose round trip and the ACT exp chain of a
block overlap with the next projection slab on the PE.
"""

import numpy as np
import ml_dtypes

import concourse.bacc as bacc
import concourse.mybir as mybir
import concourse.tile as tile
from concourse.bass_utils import run_bass_kernel_spmd

F32 = mybir.dt.float32
BF16 = mybir.dt.bfloat16

B, T, C, D = 16, 2048, 1024, 128
NCORES = 8
BLOC = B // NCORES  # batches per core
NJ = T // 512  # 512-wide q blocks
NKT = T // 128  # 128-wide k tiles
SCALE = float(D) ** -0.5


def _build():
    nc = bacc.Bacc("TRN2", target_bir_lowering=False, debug=False, num_devices=NCORES)
    x_d = nc.dram_tensor("x", [BLOC, T, C], BF16, kind="ExternalInput").ap()
    wq_d = nc.dram_tensor("Wq", [C, D], BF16, kind="ExternalInput").ap()
    wk_d = nc.dram_tensor("Wk", [C, D], BF16, kind="ExternalInput").ap()
    wv_d = nc.dram_tensor("Wv", [C, D], BF16, kind="ExternalInput").ap()
    out_d = nc.dram_tensor("out", [BLOC, T, D], F32, kind="ExternalOutput").ap()

    with tile.TileContext(nc) as tc:
        _emit(nc, tc, x_d, (wq_d, wk_d, wv_d), out_d)
    nc.compile()
    return nc


def _emit(nc, tc, x_d, w_ds, out_d):
    from contextlib import ExitStack

    ctx = ExitStack()
    with ctx:
        const = ctx.enter_context(tc.tile_pool(name="const", bufs=1))
        xtp = ctx.enter_context(tc.tile_pool(name="xtp", bufs=2))
        qkv = ctx.enter_context(tc.tile_pool(name="qkv", bufs=2))
        ptsub = ctx.enter_context(tc.tile_pool(name="ptsub", bufs=2))
        ptnat = ctx.enter_context(tc.tile_pool(name="ptnat", bufs=8))
        ptwed = ctx.enter_context(tc.tile_pool(name="ptwed", bufs=8))
        part = ctx.enter_context(tc.tile_pool(name="part", bufs=2))
        small = ctx.enter_context(tc.tile_pool(name="small", bufs=8))
        ps_proj = ctx.enter_context(tc.tile_pool(name="ps_proj", bufs=2, space="PSUM"))
        ps_s = ctx.enter_context(tc.tile_pool(name="ps_s", bufs=3, space="PSUM"))
        ps_o = ctx.enter_context(tc.tile_pool(name="ps_o", bufs=1, space="PSUM"))
        ps_rs = ctx.enter_context(tc.tile_pool(name="ps_rs", bufs=2, space="PSUM"))

        # ---- constants ----
        one_one = const.tile([1, 1], F32, tag="oneone")
        nc.vector.memset(one_one, 1.0)
        w_t = const.tile([128, 3, 8, 128], BF16, tag="w")

        # evac engine round-robin (PSUM -> SBUF copies / scaled copies)
        evac_state = [0]

        def evac(out_ap, in_ap):
            if evac_state[0] % 2 == 0:
                nc.vector.tensor_copy(out_ap, in_ap)
            else:
                nc.scalar.copy(out_ap, in_ap)
            evac_state[0] += 1

        def norm_evac(out_ap, in_ap, recip_ap):
            if evac_state[0] % 2 == 0:
                nc.vector.tensor_scalar_mul(out_ap, in_ap, recip_ap)
            else:
                nc.scalar.activation(
                    out_ap,
                    in_ap,
                    mybir.ActivationFunctionType.Copy,
                    scale=recip_ap,
                )
            evac_state[0] += 1

        def emit_xload(b, first_only=False, rest_only=False):
            """xT [128, 8, T] bf16 via DMA xbar transpose, 4 slabs."""
            if not rest_only:
                xT = xtp.tile([128, 8, T], BF16, tag="xT")
                emit_xload.cur = xT
            xT = emit_xload.cur
            slabs = range(1) if first_only else (range(1, 4) if rest_only else range(4))
            for s in slabs:
                sl = slice(512 * s, 512 * (s + 1))
                nc.sync.dma_start_transpose(xT[:, :, sl], x_d[b, sl, :])
            return xT

        def emit_wload():
            for p in range(3):
                nc.sync.dma_start(
                    out=w_t[:, p], in_=w_ds[p].rearrange("(k p) d -> p k d", p=128)
                )

        def emit_proj_slab(xT, qkv_t, j):
            """Projections for 512-wide t slab j."""
            qT, kT, V = qkv_t
            sl = slice(512 * j, 512 * (j + 1))
            for p, dst in ((0, qT), (1, kT)):
                acc = ps_proj.tile([128, 512], F32, tag="proj")
                for kk in range(8):
                    nc.tensor.matmul(
                        acc,
                        w_t[:, p, kk],
                        xT[:, kk, sl],
                        start=(kk == 0),
                        stop=(kk == 7),
                    )
                evac(dst[:, sl], acc)
            # V natural: per 128-t chunk within the slab
            acc = ps_proj.tile([128, 512], F32, tag="proj")
            for m in range(4):
                tch = 4 * j + m
                for kk in range(8):
                    nc.tensor.matmul(
                        acc[:, 128 * m : 128 * (m + 1)],
                        xT[:, kk, 128 * tch : 128 * (tch + 1)],
                        w_t[:, 2, kk],
                        start=(kk == 0),
                        stop=(kk == 7),
                    )
            evac(V[:, 4 * j : 4 * (j + 1)].rearrange("p m d -> p (m d)"), acc)

        def emit_pre(b, qkv_t, j):
            """S matmuls, exps, rowsums and wedge transposes for block j."""
            qT, kT, V = qkv_t
            nsub = 4 * j
            qsl = slice(512 * j, 512 * (j + 1))

            # ---- diagonal wedge first (longest AV-latency chain) ----
            wedges = []
            for qq in range(4):
                gq = 4 * j + qq
                nw = qq + 1
                sn = ps_s.tile([128, 512], F32, tag="s")
                for m in range(nw):
                    tk = 4 * j + m
                    nc.tensor.matmul(
                        sn[:, 128 * m : 128 * (m + 1)],
                        qT[:, 128 * gq : 128 * (gq + 1)],
                        kT[:, 128 * tk : 128 * (tk + 1)],
                        start=True,
                        stop=True,
                    )
                ptn = ptnat.tile([128, 512], BF16, tag="ptnat")
                nc.scalar.activation(
                    ptn[:, : 128 * nw],
                    sn[:, : 128 * nw],
                    mybir.ActivationFunctionType.Exp,
                    scale=SCALE,
                )
                # causal mask on the diagonal 128x128 slice: keep q >= k
                nc.gpsimd.affine_select(
                    out=ptn[:, 128 * qq : 128 * (qq + 1)],
                    in_=ptn[:, 128 * qq : 128 * (qq + 1)],
                    compare_op=mybir.AluOpType.is_ge,
                    fill=0.0,
                    base=0,
                    pattern=[[-1, 128]],
                    channel_multiplier=1,
                )
                rsw = small.tile([128, 1], F32, tag="rsw")
                nc.vector.tensor_reduce(
                    out=rsw,
                    in_=ptn[:, : 128 * nw],
                    axis=mybir.AxisListType.X,
                    op=mybir.AluOpType.add,
                )
                ptw = ptwed.tile([128, 4, 128], BF16, tag="ptwed")
                nc.sync.dma_start_transpose(ptw[:, :nw, :], ptn[:, : 128 * nw])
                wedges.append((ptw, rsw))

            # ---- sub-diagonal tiles in S^T form ----
            ptT_sub = None
            rsT = None
            if nsub:
                ptT_sub = ptsub.tile([128, 12, 512], BF16, tag="ptsub")
                partial = part.tile([1, 12, 512], F32, tag="partial")
                rsT = ps_rs.tile([128, 4], F32, tag="rsT")
            for tk in range(nsub):
                sp = ps_s.tile([128, 512], F32, tag="s")
                nc.tensor.matmul(
                    sp,
                    kT[:, 128 * tk : 128 * (tk + 1)],
                    qT[:, qsl],
                    start=True,
                    stop=True,
                )
                nc.scalar.activation(
                    ptT_sub[:, tk, :],
                    sp,
                    mybir.ActivationFunctionType.Exp,
                    scale=SCALE,
                )
                nc.gpsimd.tensor_reduce(
                    out=partial[:, tk, :],
                    in_=ptT_sub[:, tk, :],
                    axis=mybir.AxisListType.C,
                    op=mybir.AluOpType.add,
                )
                # transpose partials into rsT[:, qq] via PSUM accumulation.
                # NOTE: start=True zeroes the full psum zero-region, so a
                # single start/stop for the whole interleaved chain.
                for qq in range(4):
                    nc.tensor.matmul(
                        rsT[:, qq : qq + 1],
                        partial[0:1, tk, 128 * qq : 128 * (qq + 1)],
                        one_one,
                        start=(tk == 0 and qq == 0),
                        stop=(tk == nsub - 1 and qq == 3),
                    )
            return (wedges, ptT_sub, rsT, nsub)

        def emit_av(b, qkv_t, j, state):
            qT, kT, V = qkv_t
            wedges, ptT_sub, rsT, nsub = state
            ot = ps_o.tile([128, 512], F32, tag="ot")
            for qq in range(4):
                gq = 4 * j + qq
                ptw, rsw = wedges[qq]
                osl = ot[:, 128 * qq : 128 * (qq + 1)]
                nmm = gq + 1
                i = 0
                for tk in range(nsub):
                    nc.tensor.matmul(
                        osl,
                        ptT_sub[:, tk, 128 * qq : 128 * (qq + 1)],
                        V[:, tk, :],
                        start=(i == 0),
                        stop=(i == nmm - 1),
                    )
                    i += 1
                for m in range(qq + 1):
                    nc.tensor.matmul(
                        osl,
                        ptw[:, m, :],
                        V[:, 4 * j + m, :],
                        start=(i == 0),
                        stop=(i == nmm - 1),
                    )
                    i += 1
                recip = small.tile([128, 1], F32, tag="recip")
                if nsub:
                    tot = small.tile([128, 1], F32, tag="tot")
                    nc.vector.tensor_tensor(
                        tot, rsw, rsT[:, qq : qq + 1], mybir.AluOpType.add
                    )
                    nc.vector.reciprocal(recip, tot)
                else:
                    nc.vector.reciprocal(recip, rsw)
                o_sb = small.tile([128, 128], F32, tag="osb")
                norm_evac(o_sb, osl, recip)
                nc.sync.dma_start(
                    out=out_d[b, 128 * gq : 128 * (gq + 1), :], in_=o_sb
                )

        # ---------- schedule ----------
        def new_qkv():
            return (
                qkv.tile([128, T], BF16, tag="qT", name="qT"),
                qkv.tile([128, T], BF16, tag="kT", name="kT"),
                qkv.tile([128, NKT, 128], BF16, tag="V", name="V"),
            )

        # first x slab before the (larger) weight load so proj can start early
        xT0 = emit_xload(0, first_only=True)
        emit_wload()
        emit_xload(0, rest_only=True)
        qkv0 = new_qkv()
        xT = {0: xT0}
        qkvs = {0: qkv0}
        prev = None  # (b, qkv_t, j, state) pending AV
        for b in range(BLOC):
            for j in range(NJ):
                emit_proj_slab(xT[b], qkvs[b], j)
                st = emit_pre(b, qkvs[b], j)
                if prev is not None:
                    emit_av(*prev)
                prev = (b, qkvs[b], j, st)
                if b == 0 and j == NJ - 1:
                    xT[1] = emit_xload(1)
                    qkvs[1] = new_qkv()
        emit_av(*prev)


_NC = None


def _get_nc():
    global _NC
    if _NC is None:
        _NC = _build()
    return _NC


def _in_maps(x, Wq, Wk, Wv):
    xb = np.ascontiguousarray(x).astype(ml_dtypes.bfloat16)
    wqb = np.ascontiguousarray(Wq).astype(ml_dtypes.bfloat16)
    wkb = np.ascontiguousarray(Wk).astype(ml_dtypes.bfloat16)
    wvb = np.ascontiguousarray(Wv).astype(ml_dtypes.bfloat16)
    return [
        {"x": xb[BLOC * c : BLOC * (c + 1)], "Wq": wqb, "Wk": wkb, "Wv": wvb}
        for c in range(NCORES)
    ]


def kernel(x, Wq, Wk, Wv):
    nc = _get_nc()
    res = run_bass_kernel_spmd(nc, _in_maps(x, Wq, Wk, Wv), core_ids=list(range(NCORES)))
    return np.concatenate([res.results[c]["out"] for c in range(NCORES)], axis=0)
